# revision 46
# baseline (speedup 1.0000x reference)
"""Multi-head attention (RoPE, causal) Trainium2 Bass kernel, 8-way sharded.

Sharding: tensor-parallel over heads x data-parallel over batch.
  core c (0..7): batch b = c // 4, head group hg = c % 4 -> heads [4*hg, 4*hg+4).
Each core computes its 4 heads' QKV projection, RoPE, causal attention, and a
partial output projection (its 512 columns of the E-dim contraction).  The host
sums the 4 partials per batch and transposes back.

v2 design (vs the fp32r/spill baseline):
  * all matmul operands are bf16 (full-rate on the PE like fp32r, but half the
    DMA traffic and half the LDWEIGHTS rows); PSUM accumulation stays fp32.
  * q/k/v stay resident in SBUF after projection+RoPE - no DRAM spill.
  * softmax denominator: exp tiles are accumulated on the DVE, then a single
    all-ones [128,128] matmul per (head, query-block) computes the partition
    sum broadcast to all partitions in one 213ns PE op - replacing the 160
    per-j-tile ones-matmuls (34us of PE) the baseline used.  No gpsimd ops at
    all: any gpsimd use triggers library load/unload drain barriers (~6us,
    all engines).
  * single woven instruction stream: attention for query-block t-1 is
    interleaved into the QKV projection of block t, and the output projection
    is interleaved into the final attention block, so the PE queue never
    drains while scalar/vector engines catch up.
Softmax skips the max-subtraction (logits are O(+-10) here so exp cannot
overflow).
"""

import sys

sys.path.insert(0, "/opt/trn_rl_repo")

import numpy as np

import concourse.bass as bass  # noqa: F401
import concourse.tile as tile
from concourse import bacc, mybir
from concourse import bass_utils

try:
    import ml_dtypes
except ImportError:  # pragma: no cover
    ml_dtypes = None

# bass_utils' trace path imports antenv.axon_hooks, which may be absent from
# this image; register a no-op hook module so an externally-set BASS_TRACE
# degrades to "no profile" instead of crashing the run.
try:
    import antenv.axon_hooks  # noqa: F401
except ImportError:
    import types

    _hooks = types.ModuleType("antenv.axon_hooks")
    _hooks.get_axon_ntff_profile_hook = lambda: None
    _hooks.set_axon_ntff_profile_hook = lambda h: None
    sys.modules["antenv.axon_hooks"] = _hooks
    try:
        import antenv

        antenv.axon_hooks = _hooks
    except ImportError:
        pass

# Problem shape (hardcoded per contract).
B = 2
S = 2048
E = 2048
H = 16
D = 128
N_CORES = 8
GPB = N_CORES // B  # head groups per batch = 4
HPC = H // GPB  # heads per core = 4
DPC = HPC * D  # feature cols per core = 512
SBLK = 512
NSBLK = S // SBLK  # 4
NECH = E // 128  # 16 contraction chunks
SM_SCALE = float(D) ** -0.5

F32 = mybir.dt.float32
BF16 = mybir.dt.bfloat16

_CACHE = {}
_RUN_KWARGS = {}


def _build_nc():
    nc = bacc.Bacc(
        "TRN2",
        target_bir_lowering=False,
        debug=False,
        enable_asserts=True,
        num_devices=N_CORES,
    )
    xT = nc.dram_tensor("xT", [E, S], BF16, kind="ExternalInput").ap()
    wqkT = nc.dram_tensor("wqkT", [E, 2 * DPC], BF16, kind="ExternalInput").ap()
    wvT = nc.dram_tensor("wvT", [E, DPC], BF16, kind="ExternalInput").ap()
    woutT = nc.dram_tensor("woutT", [DPC, E], BF16, kind="ExternalInput").ap()
    cosT = nc.dram_tensor("cosT", [D, S], BF16, kind="ExternalInput").ap()
    sinTs = nc.dram_tensor("sinTs", [D, S], BF16, kind="ExternalInput").ap()
    tri = nc.dram_tensor("tri", [128, 128], BF16, kind="ExternalInput").ap()
    ones = nc.dram_tensor("ones", [128, 128], mybir.dt.float32r, kind="ExternalInput").ap()
    outT = nc.dram_tensor("outT", [E, S], F32, kind="ExternalOutput").ap()

    with tile.TileContext(nc) as tc, nc.allow_low_precision(reason="bf16 mma"):
        with (
            tc.tile_pool(name="const", bufs=1) as const_pool,
            tc.tile_pool(name="wqk", bufs=1) as wqk_pool,
            tc.tile_pool(name="wv", bufs=NECH) as wv_pool,
            tc.tile_pool(name="wo", bufs=HPC) as wo_pool,
            tc.tile_pool(name="xts", bufs=32) as xts_pool,
            tc.tile_pool(name="qk", bufs=1) as qk_pool,
            tc.tile_pool(name="vres", bufs=1) as v_pool,
            tc.tile_pool(name="ctx", bufs=1) as ctx_pool,
            tc.tile_pool(name="rtmp", bufs=2) as rtmp_pool,
            tc.tile_pool(name="ex", bufs=8) as ex_pool,
            tc.tile_pool(name="acc", bufs=2) as acc_pool,
            tc.tile_pool(name="rcp", bufs=2) as rc_pool,
            tc.tile_pool(name="osb", bufs=4) as osb_pool,
        ):
            cos_sb = const_pool.tile([D, S], BF16, tag="cos", name="cos_sb")
            sin_sb = const_pool.tile([D, S], BF16, tag="sin", name="sin_sb")
            tri_sb = const_pool.tile([128, 128], BF16, tag="tri", name="tri_sb")
            ones_sb = const_pool.tile(
                [128, 128], mybir.dt.float32r, tag="ones", name="ones_sb"
            )

            v_sb = v_pool.tile([128, (S // 128) * DPC], BF16, tag="v", name="v_sb")
            q_sb = [
                [
                    qk_pool.tile([128, SBLK], BF16, tag=f"q{h}_{t}", name=f"q{h}_{t}")
                    for t in range(NSBLK)
                ]
                for h in range(HPC)
            ]
            k_sb = [
                [
                    qk_pool.tile([128, SBLK], BF16, tag=f"k{h}_{t}", name=f"k{h}_{t}")
                    for t in range(NSBLK)
                ]
                for h in range(HPC)
            ]
            ctx16 = [
                [
                    ctx_pool.tile([128, SBLK], BF16, tag=f"c{h}_{t}", name=f"c{h}_{t}")
                    for t in range(NSBLK)
                ]
                for h in range(HPC)
            ]

            # ---------------- preloads ----------------
            # small consts first: the first RoPE (DVE) needs cos/sin and must
            # not queue behind megabytes of weight traffic
            nc.sync.dma_start(cos_sb[:], cosT[:])
            nc.sync.dma_start(sin_sb[:], sinTs[:])
            nc.sync.dma_start(tri_sb[:], tri[:])
            nc.sync.dma_start(ones_sb[:], ones[:])
            # q-half weights + x first: the q chains only need these 4MB
            wq_t = []
            wk_t = []
            xts = [[None] * NECH for _ in range(NSBLK)]
            for e in range(NECH):
                wt = wqk_pool.tile([128, DPC], BF16, tag=f"wq{e}", name="wq")
                nc.sync.dma_start(wt[:], wqkT[e * 128 : (e + 1) * 128, 0:DPC])
                wq_t.append(wt)
                xt = xts_pool.tile([128, SBLK], BF16, tag="xt", name="xt")
                nc.sync.dma_start(xt[:], xT[e * 128 : (e + 1) * 128, 0:SBLK])
                xts[0][e] = xt
            for e in range(NECH):
                wt = wqk_pool.tile([128, DPC], BF16, tag=f"wk{e}", name="wk")
                nc.sync.dma_start(wt[:], wqkT[e * 128 : (e + 1) * 128, DPC : 2 * DPC])
                wk_t.append(wt)
            wv_t = []
            for e in range(NECH):
                wt = wv_pool.tile([128, DPC], BF16, tag="wv", name="wv")
                nc.sync.dma_start(wt[:], wvT[e * 128 : (e + 1) * 128, :])
                wv_t.append(wt)
            wo_t = []

            def load_wo():
                # deferred: wo is needed only in the tail, keep the early DMA
                # queue free for the x-tile prefetch
                for h in range(HPC):
                    wt = wo_pool.tile([128, E], BF16, tag="wo", name="wo")
                    nc.sync.dma_start(wt[:], woutT[h * 128 : (h + 1) * 128, :])
                    wo_t.append(wt)

            def prefetch_x(sb):
                for e in range(NECH):
                    xt = xts_pool.tile([128, SBLK], BF16, tag="xt", name="xt")
                    nc.sync.dma_start(
                        xt[:], xT[e * 128 : (e + 1) * 128, sb * SBLK : (sb + 1) * SBLK]
                    )
                    xts[sb][e] = xt

            def rope_to(dst, ps, ssl):
                t1 = rtmp_pool.tile([128, SBLK], F32, tag="t1", name="t1")
                nc.vector.tensor_mul(t1[:], ps[:], cos_sb[:, ssl])
                t2 = rtmp_pool.tile([128, SBLK], F32, tag="t2", name="t2")
                nc.vector.tensor_mul(t2[0:64, :], ps[64:128, :], sin_sb[0:64, ssl])
                nc.vector.tensor_mul(t2[64:128, :], ps[0:64, :], sin_sb[64:128, ssl])
                nc.vector.tensor_add(dst[:], t1[:], t2[:])

            def emit_qkv_chain(pj_pool, sb, kind, m):
                """One projection chain: q/k head m, or v s-tile m, of block sb."""
                ssl = slice(sb * SBLK, (sb + 1) * SBLK)
                ps = pj_pool.tile([128, SBLK], F32, tag="pj", name="pj")
                if kind == "q" or kind == "k":
                    w_t = wq_t if kind == "q" else wk_t
                    for e in range(NECH):
                        nc.tensor.matmul(
                            ps[:],
                            w_t[e][:, m * 128 : (m + 1) * 128],
                            xts[sb][e][:],
                            start=(e == 0),
                            stop=(e == NECH - 1),
                        )
                    dst = (q_sb if kind == "q" else k_sb)[m][sb]
                    rope_to(dst, ps, ssl)
                else:  # v: natural [s, d'] layout
                    st = sb * (SBLK // 128) + m
                    for e in range(NECH):
                        nc.tensor.matmul(
                            ps[:],
                            xts[sb][e][:, m * 128 : (m + 1) * 128],
                            wv_t[e][:],
                            start=(e == 0),
                            stop=(e == NECH - 1),
                        )
                    # v-copy on the DVE: keeps the scalar queue free for exps
                    nc.vector.tensor_copy(v_sb[:, st * DPC : (st + 1) * DPC], ps[:])

            def att_row(h, t):
                """Generator: attention row (head h, query block t); yields
                after each j-tile so QKV/out-proj work can be woven in."""
                njt = 4 * (t + 1)
                ctx_ps = pctx_pool.tile([128, SBLK], F32, tag="pctx", name="pctx")
                acc = acc_pool.tile(
                    [128, SBLK], mybir.dt.float32r, tag="acc", name="acc"
                )
                ex_first = None
                inflight = []

                def emit_ctx(work):
                    jt, lo, ex = work
                    nc.tensor.matmul(
                        ctx_ps[:, lo:SBLK],
                        v_sb[:, jt * DPC + h * 128 : jt * DPC + (h + 1) * 128],
                        ex[:, lo:SBLK],
                        start=(jt == 0),
                        stop=(jt == njt - 1),
                        skip_group_check=True,
                    )

                for jt in range(njt):
                    o = jt - 4 * t
                    # causal: columns < jt*128 of this query block see none of
                    # this j-tile's keys -> shrink the moving dim
                    lo = max(o, 0) * 128
                    sc = psc_pool.tile([128, SBLK], F32, tag="sc", name="sc")
                    nc.tensor.matmul(
                        sc[:, lo:SBLK],
                        k_sb[h][jt // 4][:, (jt % 4) * 128 : (jt % 4 + 1) * 128],
                        q_sb[h][t][:, lo:SBLK],
                        start=True,
                        stop=True,
                    )
                    ex = ex_pool.tile([128, SBLK], BF16, tag="ex", name="ex")
                    nc.scalar.activation(
                        ex[:, lo:SBLK],
                        sc[:, lo:SBLK],
                        mybir.ActivationFunctionType.Exp,
                        scale=SM_SCALE,
                    )
                    if o >= 0:
                        # partial triangle block: keys > query within it.
                        # DVE (not gpsimd): any gpsimd op triggers library
                        # load/unload drain barriers that stall every engine
                        nc.vector.tensor_mul(
                            ex[:, lo : lo + 128], ex[:, lo : lo + 128], tri_sb[:]
                        )
                    # denominator accumulation on the DVE (off the PE)
                    if jt == 0:
                        ex_first = ex
                    elif jt == 1:
                        nc.vector.tensor_add(
                            acc[:, lo:SBLK], ex_first[:, lo:SBLK], ex[:, lo:SBLK]
                        )
                        if lo > 0:  # only t==0: jt=1 starts at col 128
                            nc.vector.tensor_copy(acc[:, 0:lo], ex_first[:, 0:lo])
                    else:
                        nc.vector.tensor_add(
                            acc[:, lo:SBLK], acc[:, lo:SBLK], ex[:, lo:SBLK]
                        )
                    inflight.append((jt, lo, ex))
                    if len(inflight) > 3:
                        emit_ctx(inflight.pop(0))
                    yield
                # yield before the flush: a woven chain covers the last
                # exp->mask latency so the remaining ctx matmuls never stall
                yield
                for work in inflight:
                    emit_ctx(work)
                yield
                # denominator: all-ones matmul = partition-sum broadcast to all
                # 128 partitions in one 213ns PE op (no gpsimd round trip)
                den_b = psc_pool.tile([128, SBLK], F32, tag="sc", name="den")
                nc.tensor.matmul(
                    den_b[:], ones_sb[:], acc[:], start=True, stop=True
                )
                rc = rc_pool.tile([128, SBLK], F32, tag="rc", name="rc")
                nc.vector.reciprocal_approx_fast(out=rc[:], in_=den_b[:])
                nc.vector.tensor_mul(ctx16[h][t][:], ctx_ps[:], rc[:])
                yield

            def emit_out_mchain(pout_pool, sb, m):
                ssl = slice(sb * SBLK, (sb + 1) * SBLK)
                po = pout_pool.tile([128, SBLK], F32, tag="pj", name="po")
                for h in range(HPC):
                    nc.tensor.matmul(
                        po[:],
                        wo_t[h][:, m * 128 : (m + 1) * 128],
                        ctx16[h][sb][:],
                        start=(h == 0),
                        stop=(h == HPC - 1),
                    )
                ot = osb_pool.tile([128, SBLK], F32, tag="ot", name="ot")
                # alternate copy engine so neither queue gates the PE
                if m % 2 == 0:
                    nc.scalar.copy(ot[:], po[:])
                else:
                    nc.vector.tensor_copy(ot[:], po[:])
                nc.sync.dma_start(outT[m * 128 : (m + 1) * 128, ssl], ot[:])

            def advance(gen_list, n):
                # round-robin the first two live rows: a sibling row's j-tiles
                # hide the finalize latency (den->recip->fmul) of the other
                rr = 0
                while n > 0 and gen_list:
                    idx = rr % min(2, len(gen_list))
                    rr += 1
                    try:
                        next(gen_list[idx])
                        n -= 1
                    except StopIteration:
                        gen_list.pop(idx)

            # ------------- phase A: QKV streams with woven attention -------------
            with (
                tc.tile_pool(name="ps_sc", bufs=3, space="PSUM") as psc_pool,
                tc.tile_pool(name="ps_ctx", bufs=2, space="PSUM") as pctx_pool,
                tc.tile_pool(name="ps_j", bufs=3, space="PSUM") as pj_pool,
            ):
                for sb in range(NSBLK):
                    rows = (
                        [att_row(h, sb - 1) for h in range(HPC)] if sb >= 1 else []
                    )
                    n_yields = 4 * (4 * sb + 3)
                    per = max(1, -(-n_yields // 12))  # ceil over 12 chains
                    chains = [("q", m) for m in range(HPC)]
                    chains += [("k", m) for m in range(HPC)]
                    chains += [("v", m) for m in range(SBLK // 128)]
                    for ci, (kind, m) in enumerate(chains):
                        emit_qkv_chain(pj_pool, sb, kind, m)
                        if ci == 1 and sb < NSBLK - 1:
                            prefetch_x(sb + 1)
                        if ci == 3 and sb == 0:
                            load_wo()
                        advance(rows, per)
                    advance(rows, 10**9)

                # -------- tail: att(3) woven with the output projection --------
                # out-proj chains share the (now idle) projection PSUM pool,
                # avoiding a pool-transition barrier on the tensor queue
                rows = [att_row(h, NSBLK - 1) for h in range(HPC)]
                for sb in range(NSBLK - 1):
                    for m in range(E // 128):
                        emit_out_mchain(pj_pool, sb, m)
                        advance(rows, 3)
                advance(rows, 10**9)
                for m in range(E // 128):
                    emit_out_mchain(pj_pool, NSBLK - 1, m)

    nc.compile()
    return nc


def _rope_tables():
    inv_freq = 1.0 / (10000.0 ** (np.arange(0, D, 2, dtype=np.float64) / D))
    t = np.arange(S, dtype=np.float64)
    freqs = np.outer(t, inv_freq)  # (S, D/2)
    emb = np.concatenate([freqs, freqs], axis=-1)  # (S, D)
    cosT = np.cos(emb).T.astype(np.float32).copy()  # (D, S)
    sinT = np.sin(emb).T.astype(np.float32)
    sinTs = sinT.copy()
    sinTs[: D // 2] = -sinT[: D // 2]
    return cosT, np.ascontiguousarray(sinTs)


def _numpy_fallback(x, mask, wqkv, bqkv, wout, bout):
    qkv = x @ wqkv.T + bqkv
    q, k, v = np.split(qkv, 3, axis=-1)
    q = q.reshape(B, S, H, D).transpose(0, 2, 1, 3)
    k = k.reshape(B, S, H, D).transpose(0, 2, 1, 3)
    v = v.reshape(B, S, H, D).transpose(0, 2, 1, 3)
    inv_freq = 1.0 / (10000.0 ** (np.arange(0, D, 2, dtype=np.float32) / D))
    t = np.arange(S, dtype=np.float32)
    freqs = np.outer(t, inv_freq)
    emb = np.concatenate([freqs, freqs], axis=-1)
    cos, sin = np.cos(emb), np.sin(emb)

    def rot(a):
        a1, a2 = np.split(a, 2, axis=-1)
        return np.concatenate([-a2, a1], axis=-1)

    q = q * cos + rot(q) * sin
    k = k * cos + rot(k) * sin
    scores = np.einsum("bhqd,bhkd->bhqk", q, k) * SM_SCALE
    scores = np.where(mask, -np.inf, scores)
    scores = scores - scores.max(axis=-1, keepdims=True)
    w = np.exp(scores)
    w = w / w.sum(axis=-1, keepdims=True)
    ctx = np.einsum("bhqk,bhkd->bhqd", w, v)
    ctx = ctx.transpose(0, 2, 1, 3).reshape(B, S, E)
    return (ctx @ wout.T + bout).astype(np.float32)


def kernel(x, mask, wqkv, bqkv, wout, bout, **_):
    x = np.ascontiguousarray(np.asarray(x), dtype=np.float32)
    wqkv = np.ascontiguousarray(np.asarray(wqkv), dtype=np.float32)
    bqkv = np.asarray(bqkv, dtype=np.float32)
    wout = np.ascontiguousarray(np.asarray(wout), dtype=np.float32)
    bout = np.asarray(bout, dtype=np.float32)
    mask = np.asarray(mask)

    causal = np.array_equal(mask, np.triu(np.ones((S, S), dtype=bool), k=1))
    if not causal or np.any(bqkv) or ml_dtypes is None:
        return _numpy_fallback(x, mask, wqkv, bqkv, wout, bout)

    if "nc" not in _CACHE:
        _CACHE["nc"] = _build_nc()
    nc = _CACHE["nc"]

    bf16 = ml_dtypes.bfloat16
    cosT, sinTs = _rope_tables()
    cosT = cosT.astype(bf16)
    sinTs = sinTs.astype(bf16)
    r = np.arange(128)
    tri = (r[:, None] <= r[None, :]).astype(bf16)
    ones = np.ones((128, 128), dtype=np.float32)

    in_maps = []
    for c in range(N_CORES):
        b, hg = divmod(c, GPB)
        cols = slice(hg * DPC, (hg + 1) * DPC)
        wq = wqkv[0 * E : 1 * E, :][cols, :]  # (512, E)
        wk = wqkv[1 * E : 2 * E, :][cols, :]
        wv = wqkv[2 * E : 3 * E, :][cols, :]
        in_maps.append(
            {
                "xT": np.ascontiguousarray(x[b].T).astype(bf16),
                "wqkT": np.ascontiguousarray(
                    np.concatenate([wq, wk], axis=0).T
                ).astype(bf16),
                "wvT": np.ascontiguousarray(wv.T).astype(bf16),
                "woutT": np.ascontiguousarray(wout[:, cols].T).astype(bf16),
                "cosT": cosT,
                "sinTs": sinTs,
                "tri": tri,
                "ones": ones,
            }
        )

    res = bass_utils.run_bass_kernel_spmd(
        nc, in_maps, core_ids=list(range(N_CORES)), **_RUN_KWARGS
    )
    _CACHE["last_results"] = res

    out = np.empty((B, S, E), dtype=np.float32)
    for b in range(B):
        acc = res.results[b * GPB]["outT"].copy()
        for g in range(1, GPB):
            acc += res.results[b * GPB + g]["outT"]
        out[b] = acc.T
    out += bout
    return out


# revision 50
# speedup vs baseline: 1.1904x; 1.1904x over previous
"""Multi-head attention (RoPE, causal) Trainium2 Bass kernel, 8-way sharded.

Sharding: tensor-parallel over heads x data-parallel over batch.
  core c (0..7): batch b = c // 4, head group hg = c % 4 -> heads [4*hg, 4*hg+4).
Each core computes its 4 heads' QKV projection, RoPE, causal attention, and a
partial output projection (its 512 columns of the E-dim contraction).  The host
sums the 4 partials per batch and transposes back.

v2 design (vs the fp32r/spill baseline):
  * all matmul operands are bf16 (full-rate on the PE like fp32r, but half the
    DMA traffic and half the LDWEIGHTS rows); PSUM accumulation stays fp32.
  * q/k/v stay resident in SBUF after projection+RoPE - no DRAM spill.
  * softmax denominator: exp tiles are accumulated on the DVE, then a single
    all-ones [128,128] matmul per (head, query-block) computes the partition
    sum broadcast to all partitions in one 213ns PE op - replacing the 160
    per-j-tile ones-matmuls (34us of PE) the baseline used.  No gpsimd ops at
    all: any gpsimd use triggers library load/unload drain barriers (~6us,
    all engines).
  * single woven instruction stream: attention for query-block t-1 is
    interleaved into the QKV projection of block t, and the output projection
    is interleaved into the final attention block, so the PE queue never
    drains while scalar/vector engines catch up.
Softmax skips the max-subtraction (logits are O(+-10) here so exp cannot
overflow).
"""

import sys

sys.path.insert(0, "/opt/trn_rl_repo")

import numpy as np

import concourse.bass as bass  # noqa: F401
import concourse.tile as tile
from concourse import bacc, mybir
from concourse import bass_utils

try:
    import ml_dtypes
except ImportError:  # pragma: no cover
    ml_dtypes = None

# bass_utils' trace path imports antenv.axon_hooks, which may be absent from
# this image; register a no-op hook module so an externally-set BASS_TRACE
# degrades to "no profile" instead of crashing the run.
try:
    import antenv.axon_hooks  # noqa: F401
except ImportError:
    import types

    _hooks = types.ModuleType("antenv.axon_hooks")
    _hooks.get_axon_ntff_profile_hook = lambda: None
    _hooks.set_axon_ntff_profile_hook = lambda h: None
    sys.modules["antenv.axon_hooks"] = _hooks
    try:
        import antenv

        antenv.axon_hooks = _hooks
    except ImportError:
        pass

# Problem shape (hardcoded per contract).
B = 2
S = 2048
E = 2048
H = 16
D = 128
N_CORES = 8
GPB = N_CORES // B  # head groups per batch = 4
HPC = H // GPB  # heads per core = 4
DPC = HPC * D  # feature cols per core = 512
SBLK = 512
NSBLK = S // SBLK  # 4
NECH = E // 128  # 16 contraction chunks
SM_SCALE = float(D) ** -0.5

F32 = mybir.dt.float32
BF16 = mybir.dt.bfloat16

_CACHE = {}
_RUN_KWARGS = {}


def _build_nc():
    nc = bacc.Bacc(
        "TRN2",
        target_bir_lowering=False,
        debug=False,
        enable_asserts=True,
        num_devices=N_CORES,
    )
    xT = nc.dram_tensor("xT", [E, S], BF16, kind="ExternalInput").ap()
    wqkT = nc.dram_tensor("wqkT", [E, 2 * DPC], BF16, kind="ExternalInput").ap()
    wvT = nc.dram_tensor("wvT", [E, DPC], BF16, kind="ExternalInput").ap()
    woutT = nc.dram_tensor("woutT", [DPC, E], BF16, kind="ExternalInput").ap()
    cosT = nc.dram_tensor("cosT", [D, S], BF16, kind="ExternalInput").ap()
    sinTs = nc.dram_tensor("sinTs", [D, S], BF16, kind="ExternalInput").ap()
    tri = nc.dram_tensor("tri", [128, 128], BF16, kind="ExternalInput").ap()
    ones = nc.dram_tensor("ones", [128, 128], mybir.dt.float32r, kind="ExternalInput").ap()
    outT = nc.dram_tensor("outT", [E, S], F32, kind="ExternalOutput").ap()

    with tile.TileContext(nc) as tc, nc.allow_low_precision(reason="bf16 mma"):
        with (
            tc.tile_pool(name="const", bufs=1) as const_pool,
            tc.tile_pool(name="wqk", bufs=1) as wqk_pool,
            tc.tile_pool(name="wv", bufs=NECH) as wv_pool,
            tc.tile_pool(name="wo", bufs=HPC) as wo_pool,
            tc.tile_pool(name="xts", bufs=32) as xts_pool,
            tc.tile_pool(name="qk", bufs=1) as qk_pool,
            tc.tile_pool(name="vres", bufs=1) as v_pool,
            tc.tile_pool(name="ctx", bufs=1) as ctx_pool,
            tc.tile_pool(name="rtmp", bufs=2) as rtmp_pool,
            tc.tile_pool(name="ex", bufs=8) as ex_pool,
            tc.tile_pool(name="acc", bufs=2) as acc_pool,
            tc.tile_pool(name="rcp", bufs=2) as rc_pool,
            tc.tile_pool(name="osb", bufs=4) as osb_pool,
        ):
            cos_sb = const_pool.tile([D, S], BF16, tag="cos", name="cos_sb")
            sin_sb = const_pool.tile([D, S], BF16, tag="sin", name="sin_sb")
            tri_sb = const_pool.tile([128, 128], BF16, tag="tri", name="tri_sb")
            ones_sb = const_pool.tile(
                [128, 128], mybir.dt.float32r, tag="ones", name="ones_sb"
            )

            v_sb = v_pool.tile([128, (S // 128) * DPC], BF16, tag="v", name="v_sb")
            q_sb = [
                [
                    qk_pool.tile([128, SBLK], BF16, tag=f"q{h}_{t}", name=f"q{h}_{t}")
                    for t in range(NSBLK)
                ]
                for h in range(HPC)
            ]
            k_sb = [
                [
                    qk_pool.tile([128, SBLK], BF16, tag=f"k{h}_{t}", name=f"k{h}_{t}")
                    for t in range(NSBLK)
                ]
                for h in range(HPC)
            ]
            ctx16 = [
                [
                    ctx_pool.tile([128, SBLK], BF16, tag=f"c{h}_{t}", name=f"c{h}_{t}")
                    for t in range(NSBLK)
                ]
                for h in range(HPC)
            ]

            # ---------------- preloads ----------------
            # the very first matmul's deps (wq[0..1], x[0..1]) go FIRST: cold
            # DMA rate is poor, and the consts are not needed until the first
            # rope (~14us in).  Then consts, then the rest of the q-half + x.
            wq_t = []
            wk_t = []
            xts = [[None] * NECH for _ in range(NSBLK)]
            for e in range(NECH):
                wt = wqk_pool.tile([128, DPC], BF16, tag=f"wq{e}", name="wq")
                nc.sync.dma_start(wt[:], wqkT[e * 128 : (e + 1) * 128, 0:DPC])
                wq_t.append(wt)
                xt = xts_pool.tile([128, SBLK], BF16, tag="xt", name="xt")
                nc.sync.dma_start(xt[:], xT[e * 128 : (e + 1) * 128, 0:SBLK])
                xts[0][e] = xt
                if e == 1:
                    nc.sync.dma_start(cos_sb[:], cosT[:])
                    nc.sync.dma_start(sin_sb[:], sinTs[:])
                    nc.sync.dma_start(tri_sb[:], tri[:])
                    nc.sync.dma_start(ones_sb[:], ones[:])
            for e in range(NECH):
                wt = wqk_pool.tile([128, DPC], BF16, tag=f"wk{e}", name="wk")
                nc.sync.dma_start(wt[:], wqkT[e * 128 : (e + 1) * 128, DPC : 2 * DPC])
                wk_t.append(wt)
            wv_t = []
            for e in range(NECH):
                wt = wv_pool.tile([128, DPC], BF16, tag="wv", name="wv")
                nc.sync.dma_start(wt[:], wvT[e * 128 : (e + 1) * 128, :])
                wv_t.append(wt)
            wo_t = []

            def load_wo():
                # deferred: wo is needed only in the tail, keep the early DMA
                # queue free for the x-tile prefetch
                for h in range(HPC):
                    wt = wo_pool.tile([128, E], BF16, tag="wo", name="wo")
                    nc.sync.dma_start(wt[:], woutT[h * 128 : (h + 1) * 128, :])
                    wo_t.append(wt)

            def prefetch_x(sb):
                for e in range(NECH):
                    xt = xts_pool.tile([128, SBLK], BF16, tag="xt", name="xt")
                    nc.sync.dma_start(
                        xt[:], xT[e * 128 : (e + 1) * 128, sb * SBLK : (sb + 1) * SBLK]
                    )
                    xts[sb][e] = xt

            def rope_to(dst, ps, ssl):
                t1 = rtmp_pool.tile([128, SBLK], F32, tag="t1", name="t1")
                nc.vector.tensor_mul(t1[:], ps[:], cos_sb[:, ssl])
                t2 = rtmp_pool.tile([128, SBLK], F32, tag="t2", name="t2")
                nc.vector.tensor_mul(t2[0:64, :], ps[64:128, :], sin_sb[0:64, ssl])
                nc.vector.tensor_mul(t2[64:128, :], ps[0:64, :], sin_sb[64:128, ssl])
                nc.vector.tensor_add(dst[:], t1[:], t2[:])

            def emit_qkv_chain(pj_pool, sb, kind, m):
                """One projection chain: q/k head m, or v s-tile m, of block sb."""
                ssl = slice(sb * SBLK, (sb + 1) * SBLK)
                ps = pj_pool.tile([128, SBLK], F32, tag="pj", name="pj")
                if kind == "q" or kind == "k":
                    w_t = wq_t if kind == "q" else wk_t
                    for e in range(NECH):
                        nc.tensor.matmul(
                            ps[:],
                            w_t[e][:, m * 128 : (m + 1) * 128],
                            xts[sb][e][:],
                            start=(e == 0),
                            stop=(e == NECH - 1),
                        )
                    dst = (q_sb if kind == "q" else k_sb)[m][sb]
                    rope_to(dst, ps, ssl)
                else:  # v: natural [s, d'] layout
                    st = sb * (SBLK // 128) + m
                    for e in range(NECH):
                        nc.tensor.matmul(
                            ps[:],
                            xts[sb][e][:, m * 128 : (m + 1) * 128],
                            wv_t[e][:],
                            start=(e == 0),
                            stop=(e == NECH - 1),
                        )
                    # v-copy on the DVE: keeps the scalar queue free for exps
                    nc.vector.tensor_copy(v_sb[:, st * DPC : (st + 1) * DPC], ps[:])

            def att_row(h, t):
                """Generator: attention row (head h, query block t); yields
                after each j-tile so QKV/out-proj work can be woven in."""
                njt = 4 * (t + 1)
                win = 2 if t == 0 else 3  # tiny t=0 rows: pop ctx earlier
                ctx_ps = pctx_pool.tile([128, SBLK], F32, tag="pctx", name="pctx")
                acc = acc_pool.tile(
                    [128, SBLK], mybir.dt.float32r, tag="acc", name="acc"
                )
                ex_first = None
                inflight = []

                def emit_ctx(work):
                    jt, lo, ex = work
                    nc.tensor.matmul(
                        ctx_ps[:, lo:SBLK],
                        v_sb[:, jt * DPC + h * 128 : jt * DPC + (h + 1) * 128],
                        ex[:, lo:SBLK],
                        start=(jt == 0),
                        stop=(jt == njt - 1),
                        skip_group_check=True,
                    )

                for jt in range(njt):
                    o = jt - 4 * t
                    # causal: columns < jt*128 of this query block see none of
                    # this j-tile's keys -> shrink the moving dim
                    lo = max(o, 0) * 128
                    sc = psc_pool.tile([128, SBLK], F32, tag="sc", name="sc")
                    nc.tensor.matmul(
                        sc[:, lo:SBLK],
                        k_sb[h][jt // 4][:, (jt % 4) * 128 : (jt % 4 + 1) * 128],
                        q_sb[h][t][:, lo:SBLK],
                        start=True,
                        stop=True,
                    )
                    ex = ex_pool.tile([128, SBLK], BF16, tag="ex", name="ex")
                    nc.scalar.activation(
                        ex[:, lo:SBLK],
                        sc[:, lo:SBLK],
                        mybir.ActivationFunctionType.Exp,
                        scale=SM_SCALE,
                    )
                    if o >= 0:
                        # partial triangle block: keys > query within it.
                        # DVE (not gpsimd): any gpsimd op triggers library
                        # load/unload drain barriers that stall every engine
                        nc.vector.tensor_mul(
                            ex[:, lo : lo + 128], ex[:, lo : lo + 128], tri_sb[:]
                        )
                    # denominator accumulation on the DVE (off the PE)
                    if jt == 0:
                        ex_first = ex
                    elif jt == 1:
                        nc.vector.tensor_add(
                            acc[:, lo:SBLK], ex_first[:, lo:SBLK], ex[:, lo:SBLK]
                        )
                        if lo > 0:  # only t==0: jt=1 starts at col 128
                            nc.vector.tensor_copy(acc[:, 0:lo], ex_first[:, 0:lo])
                    else:
                        nc.vector.tensor_add(
                            acc[:, lo:SBLK], acc[:, lo:SBLK], ex[:, lo:SBLK]
                        )
                    inflight.append((jt, lo, ex))
                    if len(inflight) > win:
                        emit_ctx(inflight.pop(0))
                    yield
                # yield before the flush: a woven chain covers the last
                # exp->mask latency so the remaining ctx matmuls never stall
                yield
                for work in inflight:
                    emit_ctx(work)
                yield
                # denominator: all-ones matmul = partition-sum broadcast to all
                # 128 partitions in one 213ns PE op (no gpsimd round trip)
                den_b = psc_pool.tile([128, SBLK], F32, tag="sc", name="den")
                nc.tensor.matmul(
                    den_b[:], ones_sb[:], acc[:], start=True, stop=True
                )
                rc = rc_pool.tile([128, SBLK], F32, tag="rc", name="rc")
                nc.vector.reciprocal_approx_fast(out=rc[:], in_=den_b[:])
                nc.vector.tensor_mul(ctx16[h][t][:], ctx_ps[:], rc[:])
                yield

            def emit_out_mchain(pout_pool, sb, m):
                ssl = slice(sb * SBLK, (sb + 1) * SBLK)
                po = pout_pool.tile([128, SBLK], F32, tag="pj", name="po")
                for h in range(HPC):
                    nc.tensor.matmul(
                        po[:],
                        wo_t[h][:, m * 128 : (m + 1) * 128],
                        ctx16[h][sb][:],
                        start=(h == 0),
                        stop=(h == HPC - 1),
                    )
                ot = osb_pool.tile([128, SBLK], F32, tag="ot", name="ot")
                # alternate copy engine so neither queue gates the PE
                if m % 2 == 0:
                    nc.scalar.copy(ot[:], po[:])
                else:
                    nc.vector.tensor_copy(ot[:], po[:])
                nc.sync.dma_start(outT[m * 128 : (m + 1) * 128, ssl], ot[:])

            def advance(gen_list, n):
                # round-robin the first two live rows: a sibling row's j-tiles
                # hide the finalize latency (den->recip->fmul) of the other
                rr = 0
                while n > 0 and gen_list:
                    idx = rr % min(2, len(gen_list))
                    rr += 1
                    try:
                        next(gen_list[idx])
                        n -= 1
                    except StopIteration:
                        gen_list.pop(idx)

            # ------------- phase A: QKV streams with woven attention -------------
            with (
                tc.tile_pool(name="ps_sc", bufs=3, space="PSUM") as psc_pool,
                tc.tile_pool(name="ps_ctx", bufs=2, space="PSUM") as pctx_pool,
                tc.tile_pool(name="ps_j", bufs=3, space="PSUM") as pj_pool,
            ):
                # stream 0 ramp: interleave q0+q1 e-wise so the PE issues two
                # matmuls per arriving (wq[e], x[e]) pair while DMA-fed
                ps0 = pj_pool.tile([128, SBLK], F32, tag="pj", name="pj")
                ps1 = pj_pool.tile([128, SBLK], F32, tag="pj", name="pj")
                for e in range(NECH):
                    for m, ps in ((0, ps0), (1, ps1)):
                        nc.tensor.matmul(
                            ps[:],
                            wq_t[e][:, m * 128 : (m + 1) * 128],
                            xts[0][e][:],
                            start=(e == 0),
                            stop=(e == NECH - 1),
                            skip_group_check=True,
                        )
                rope_to(q_sb[0][0], ps0, slice(0, SBLK))
                rope_to(q_sb[1][0], ps1, slice(0, SBLK))

                for sb in range(NSBLK):
                    rows = (
                        [att_row(h, sb - 1) for h in range(HPC)] if sb >= 1 else []
                    )
                    n_yields = 4 * (4 * sb + 3)
                    per = max(1, -(-n_yields // 12))  # ceil over 12 chains
                    chains = [("q", m) for m in range(HPC)]
                    chains += [("k", m) for m in range(HPC)]
                    chains += [("v", m) for m in range(SBLK // 128)]
                    if sb == 0:
                        chains = chains[2:]  # q0, q1 already emitted above
                    for ci, (kind, m) in enumerate(chains):
                        emit_qkv_chain(pj_pool, sb, kind, m)
                        if ci == 1 and sb < NSBLK - 1:
                            prefetch_x(sb + 1)
                        if ci == 3 and sb == 0:
                            load_wo()
                        advance(rows, per)
                    advance(rows, 10**9)

                # -------- tail: att(3) woven with the output projection --------
                # out-proj chains share the (now idle) projection PSUM pool,
                # avoiding a pool-transition barrier on the tensor queue
                rows = [att_row(h, NSBLK - 1) for h in range(HPC)]
                for sb in range(NSBLK - 1):
                    for m in range(E // 128):
                        emit_out_mchain(pj_pool, sb, m)
                        advance(rows, 3)
                advance(rows, 10**9)
                for m in range(E // 128):
                    emit_out_mchain(pj_pool, NSBLK - 1, m)

    nc.compile()
    return nc


def _rope_tables():
    inv_freq = 1.0 / (10000.0 ** (np.arange(0, D, 2, dtype=np.float64) / D))
    t = np.arange(S, dtype=np.float64)
    freqs = np.outer(t, inv_freq)  # (S, D/2)
    emb = np.concatenate([freqs, freqs], axis=-1)  # (S, D)
    cosT = np.cos(emb).T.astype(np.float32).copy()  # (D, S)
    sinT = np.sin(emb).T.astype(np.float32)
    sinTs = sinT.copy()
    sinTs[: D // 2] = -sinT[: D // 2]
    return cosT, np.ascontiguousarray(sinTs)


def _numpy_fallback(x, mask, wqkv, bqkv, wout, bout):
    qkv = x @ wqkv.T + bqkv
    q, k, v = np.split(qkv, 3, axis=-1)
    q = q.reshape(B, S, H, D).transpose(0, 2, 1, 3)
    k = k.reshape(B, S, H, D).transpose(0, 2, 1, 3)
    v = v.reshape(B, S, H, D).transpose(0, 2, 1, 3)
    inv_freq = 1.0 / (10000.0 ** (np.arange(0, D, 2, dtype=np.float32) / D))
    t = np.arange(S, dtype=np.float32)
    freqs = np.outer(t, inv_freq)
    emb = np.concatenate([freqs, freqs], axis=-1)
    cos, sin = np.cos(emb), np.sin(emb)

    def rot(a):
        a1, a2 = np.split(a, 2, axis=-1)
        return np.concatenate([-a2, a1], axis=-1)

    q = q * cos + rot(q) * sin
    k = k * cos + rot(k) * sin
    scores = np.einsum("bhqd,bhkd->bhqk", q, k) * SM_SCALE
    scores = np.where(mask, -np.inf, scores)
    scores = scores - scores.max(axis=-1, keepdims=True)
    w = np.exp(scores)
    w = w / w.sum(axis=-1, keepdims=True)
    ctx = np.einsum("bhqk,bhkd->bhqd", w, v)
    ctx = ctx.transpose(0, 2, 1, 3).reshape(B, S, E)
    return (ctx @ wout.T + bout).astype(np.float32)


def kernel(x, mask, wqkv, bqkv, wout, bout, **_):
    x = np.ascontiguousarray(np.asarray(x), dtype=np.float32)
    wqkv = np.ascontiguousarray(np.asarray(wqkv), dtype=np.float32)
    bqkv = np.asarray(bqkv, dtype=np.float32)
    wout = np.ascontiguousarray(np.asarray(wout), dtype=np.float32)
    bout = np.asarray(bout, dtype=np.float32)
    mask = np.asarray(mask)

    causal = np.array_equal(mask, np.triu(np.ones((S, S), dtype=bool), k=1))
    if not causal or np.any(bqkv) or ml_dtypes is None:
        return _numpy_fallback(x, mask, wqkv, bqkv, wout, bout)

    if "nc" not in _CACHE:
        _CACHE["nc"] = _build_nc()
    nc = _CACHE["nc"]

    bf16 = ml_dtypes.bfloat16
    cosT, sinTs = _rope_tables()
    cosT = cosT.astype(bf16)
    sinTs = sinTs.astype(bf16)
    r = np.arange(128)
    tri = (r[:, None] <= r[None, :]).astype(bf16)
    ones = np.ones((128, 128), dtype=np.float32)

    in_maps = []
    for c in range(N_CORES):
        b, hg = divmod(c, GPB)
        cols = slice(hg * DPC, (hg + 1) * DPC)
        wq = wqkv[0 * E : 1 * E, :][cols, :]  # (512, E)
        wk = wqkv[1 * E : 2 * E, :][cols, :]
        wv = wqkv[2 * E : 3 * E, :][cols, :]
        in_maps.append(
            {
                "xT": np.ascontiguousarray(x[b].T).astype(bf16),
                "wqkT": np.ascontiguousarray(
                    np.concatenate([wq, wk], axis=0).T
                ).astype(bf16),
                "wvT": np.ascontiguousarray(wv.T).astype(bf16),
                "woutT": np.ascontiguousarray(wout[:, cols].T).astype(bf16),
                "cosT": cosT,
                "sinTs": sinTs,
                "tri": tri,
                "ones": ones,
            }
        )

    res = bass_utils.run_bass_kernel_spmd(
        nc, in_maps, core_ids=list(range(N_CORES)), **_RUN_KWARGS
    )
    _CACHE["last_results"] = res

    out = np.empty((B, S, E), dtype=np.float32)
    for b in range(B):
        acc = res.results[b * GPB]["outT"].copy()
        for g in range(1, GPB):
            acc += res.results[b * GPB + g]["outT"]
        out[b] = acc.T
    out += bout
    return out


# revision 53
# speedup vs baseline: 1.1919x; 1.0013x over previous
"""Multi-head attention (RoPE, causal) Trainium2 Bass kernel, 8-way sharded.

Sharding: tensor-parallel over heads x data-parallel over batch.
  core c (0..7): batch b = c // 4, head group hg = c % 4 -> heads [4*hg, 4*hg+4).
Each core computes its 4 heads' QKV projection, RoPE, causal attention, and a
partial output projection (its 512 columns of the E-dim contraction).  The host
sums the 4 partials per batch and transposes back.

v2 design (vs the fp32r/spill baseline):
  * all matmul operands are bf16 (full-rate on the PE like fp32r, but half the
    DMA traffic and half the LDWEIGHTS rows); PSUM accumulation stays fp32.
  * q/k/v stay resident in SBUF after projection+RoPE - no DRAM spill.
  * softmax denominator: exp tiles are accumulated on the DVE, then a single
    all-ones [128,128] matmul per (head, query-block) computes the partition
    sum broadcast to all partitions in one 213ns PE op - replacing the 160
    per-j-tile ones-matmuls (34us of PE) the baseline used.  No gpsimd ops at
    all: any gpsimd use triggers library load/unload drain barriers (~6us,
    all engines).
  * single woven instruction stream: attention for query-block t-1 is
    interleaved into the QKV projection of block t, and the output projection
    is interleaved into the final attention block, so the PE queue never
    drains while scalar/vector engines catch up.
Softmax skips the max-subtraction (logits are O(+-10) here so exp cannot
overflow).
"""

import sys

sys.path.insert(0, "/opt/trn_rl_repo")

import numpy as np

import concourse.bass as bass  # noqa: F401
import concourse.tile as tile
from concourse import bacc, mybir
from concourse import bass_utils

try:
    import ml_dtypes
except ImportError:  # pragma: no cover
    ml_dtypes = None

# bass_utils' trace path imports antenv.axon_hooks, which may be absent from
# this image; register a no-op hook module so an externally-set BASS_TRACE
# degrades to "no profile" instead of crashing the run.
try:
    import antenv.axon_hooks  # noqa: F401
except ImportError:
    import types

    _hooks = types.ModuleType("antenv.axon_hooks")
    _hooks.get_axon_ntff_profile_hook = lambda: None
    _hooks.set_axon_ntff_profile_hook = lambda h: None
    sys.modules["antenv.axon_hooks"] = _hooks
    try:
        import antenv

        antenv.axon_hooks = _hooks
    except ImportError:
        pass

# Problem shape (hardcoded per contract).
B = 2
S = 2048
E = 2048
H = 16
D = 128
N_CORES = 8
GPB = N_CORES // B  # head groups per batch = 4
HPC = H // GPB  # heads per core = 4
DPC = HPC * D  # feature cols per core = 512
SBLK = 512
NSBLK = S // SBLK  # 4
NECH = E // 128  # 16 contraction chunks
SM_SCALE = float(D) ** -0.5

F32 = mybir.dt.float32
BF16 = mybir.dt.bfloat16

_CACHE = {}
_RUN_KWARGS = {}


def _build_nc():
    nc = bacc.Bacc(
        "TRN2",
        target_bir_lowering=False,
        debug=False,
        enable_asserts=True,
        num_devices=N_CORES,
    )
    xT = nc.dram_tensor("xT", [E, S], BF16, kind="ExternalInput").ap()
    wqkT = nc.dram_tensor("wqkT", [E, 2 * DPC], BF16, kind="ExternalInput").ap()
    wvT = nc.dram_tensor("wvT", [E, DPC], BF16, kind="ExternalInput").ap()
    woutT = nc.dram_tensor("woutT", [DPC, E], BF16, kind="ExternalInput").ap()
    cosT = nc.dram_tensor("cosT", [D, S], BF16, kind="ExternalInput").ap()
    sinTs = nc.dram_tensor("sinTs", [D, S], BF16, kind="ExternalInput").ap()
    tri = nc.dram_tensor("tri", [128, 128], BF16, kind="ExternalInput").ap()
    ones = nc.dram_tensor("ones", [128, 128], mybir.dt.float32r, kind="ExternalInput").ap()
    outT = nc.dram_tensor("outT", [E, S], F32, kind="ExternalOutput").ap()

    with tile.TileContext(nc) as tc, nc.allow_low_precision(reason="bf16 mma"):
        with (
            tc.tile_pool(name="const", bufs=1) as const_pool,
            tc.tile_pool(name="wqk", bufs=1) as wqk_pool,
            tc.tile_pool(name="wv", bufs=NECH) as wv_pool,
            tc.tile_pool(name="wo", bufs=HPC) as wo_pool,
            tc.tile_pool(name="xts", bufs=32) as xts_pool,
            tc.tile_pool(name="qk", bufs=1) as qk_pool,
            tc.tile_pool(name="vres", bufs=1) as v_pool,
            tc.tile_pool(name="ctx", bufs=1) as ctx_pool,
            tc.tile_pool(name="rtmp", bufs=2) as rtmp_pool,
            tc.tile_pool(name="ex", bufs=8) as ex_pool,
            tc.tile_pool(name="acc", bufs=2) as acc_pool,
            tc.tile_pool(name="rcp", bufs=2) as rc_pool,
            tc.tile_pool(name="osb", bufs=4) as osb_pool,
        ):
            cos_sb = const_pool.tile([D, S], BF16, tag="cos", name="cos_sb")
            sin_sb = const_pool.tile([D, S], BF16, tag="sin", name="sin_sb")
            tri_sb = const_pool.tile([128, 128], BF16, tag="tri", name="tri_sb")
            ones_sb = const_pool.tile(
                [128, 128], mybir.dt.float32r, tag="ones", name="ones_sb"
            )

            v_sb = v_pool.tile([128, (S // 128) * DPC], BF16, tag="v", name="v_sb")
            q_sb = [
                [
                    qk_pool.tile([128, SBLK], BF16, tag=f"q{h}_{t}", name=f"q{h}_{t}")
                    for t in range(NSBLK)
                ]
                for h in range(HPC)
            ]
            k_sb = [
                [
                    qk_pool.tile([128, SBLK], BF16, tag=f"k{h}_{t}", name=f"k{h}_{t}")
                    for t in range(NSBLK)
                ]
                for h in range(HPC)
            ]
            ctx16 = [
                [
                    ctx_pool.tile([128, SBLK], BF16, tag=f"c{h}_{t}", name=f"c{h}_{t}")
                    for t in range(NSBLK)
                ]
                for h in range(HPC)
            ]

            # ---------------- preloads ----------------
            # the very first matmul's deps (wq[0..1], x[0..1]) go FIRST: cold
            # DMA rate is poor, and the consts are not needed until the first
            # rope (~14us in).  Then consts, then the rest of the q-half + x.
            wq_t = []
            wk_t = []
            xts = [[None] * NECH for _ in range(NSBLK)]
            for e in range(NECH):
                wt = wqk_pool.tile([128, DPC], BF16, tag=f"wq{e}", name="wq")
                nc.sync.dma_start(wt[:], wqkT[e * 128 : (e + 1) * 128, 0:DPC])
                wq_t.append(wt)
                xt = xts_pool.tile([128, SBLK], BF16, tag="xt", name="xt")
                nc.sync.dma_start(xt[:], xT[e * 128 : (e + 1) * 128, 0:SBLK])
                xts[0][e] = xt
                if e == 1:
                    nc.sync.dma_start(cos_sb[:], cosT[:])
                    nc.sync.dma_start(sin_sb[:], sinTs[:])
                    nc.sync.dma_start(tri_sb[:], tri[:])
                    nc.sync.dma_start(ones_sb[:], ones[:])
            for e in range(NECH):
                wt = wqk_pool.tile([128, DPC], BF16, tag=f"wk{e}", name="wk")
                nc.sync.dma_start(wt[:], wqkT[e * 128 : (e + 1) * 128, DPC : 2 * DPC])
                wk_t.append(wt)
            wv_t = []
            for e in range(NECH):
                wt = wv_pool.tile([128, DPC], BF16, tag="wv", name="wv")
                nc.sync.dma_start(wt[:], wvT[e * 128 : (e + 1) * 128, :])
                wv_t.append(wt)
            wo_t = []

            def load_wo():
                # deferred: wo is needed only in the tail, keep the early DMA
                # queue free for the x-tile prefetch
                for h in range(HPC):
                    wt = wo_pool.tile([128, E], BF16, tag="wo", name="wo")
                    nc.sync.dma_start(wt[:], woutT[h * 128 : (h + 1) * 128, :])
                    wo_t.append(wt)

            def prefetch_x(sb):
                for e in range(NECH):
                    xt = xts_pool.tile([128, SBLK], BF16, tag="xt", name="xt")
                    nc.sync.dma_start(
                        xt[:], xT[e * 128 : (e + 1) * 128, sb * SBLK : (sb + 1) * SBLK]
                    )
                    xts[sb][e] = xt

            def rope_to(dst, ps, ssl):
                t1 = rtmp_pool.tile([128, SBLK], F32, tag="t1", name="t1")
                nc.vector.tensor_mul(t1[:], ps[:], cos_sb[:, ssl])
                t2 = rtmp_pool.tile([128, SBLK], F32, tag="t2", name="t2")
                nc.vector.tensor_mul(t2[0:64, :], ps[64:128, :], sin_sb[0:64, ssl])
                nc.vector.tensor_mul(t2[64:128, :], ps[0:64, :], sin_sb[64:128, ssl])
                nc.vector.tensor_add(dst[:], t1[:], t2[:])

            def emit_qkv_chain(pj_pool, sb, kind, m):
                """One projection chain: q/k head m, or v s-tile m, of block sb."""
                ssl = slice(sb * SBLK, (sb + 1) * SBLK)
                ps = pj_pool.tile([128, SBLK], F32, tag="pj", name="pj")
                if kind == "q" or kind == "k":
                    w_t = wq_t if kind == "q" else wk_t
                    for e in range(NECH):
                        nc.tensor.matmul(
                            ps[:],
                            w_t[e][:, m * 128 : (m + 1) * 128],
                            xts[sb][e][:],
                            start=(e == 0),
                            stop=(e == NECH - 1),
                        )
                    dst = (q_sb if kind == "q" else k_sb)[m][sb]
                    rope_to(dst, ps, ssl)
                else:  # v: natural [s, d'] layout
                    st = sb * (SBLK // 128) + m
                    for e in range(NECH):
                        nc.tensor.matmul(
                            ps[:],
                            xts[sb][e][:, m * 128 : (m + 1) * 128],
                            wv_t[e][:],
                            start=(e == 0),
                            stop=(e == NECH - 1),
                        )
                    # v-copy on the DVE: keeps the scalar queue free for exps
                    nc.vector.tensor_copy(v_sb[:, st * DPC : (st + 1) * DPC], ps[:])

            def att_row(h, t):
                """Generator: attention row (head h, query block t); yields
                after each j-tile so QKV/out-proj work can be woven in."""
                njt = 4 * (t + 1)
                win = 2 if t == 0 else 3  # tiny t=0 rows: pop ctx earlier
                ctx_ps = pctx_pool.tile([128, SBLK], F32, tag="pctx", name="pctx")
                acc = acc_pool.tile(
                    [128, SBLK], mybir.dt.float32r, tag="acc", name="acc"
                )
                ex_first = None
                inflight = []

                def emit_ctx(work):
                    jt, lo, ex = work
                    nc.tensor.matmul(
                        ctx_ps[:, lo:SBLK],
                        v_sb[:, jt * DPC + h * 128 : jt * DPC + (h + 1) * 128],
                        ex[:, lo:SBLK],
                        start=(jt == 0),
                        stop=(jt == njt - 1),
                        skip_group_check=True,
                    )

                for jt in range(njt):
                    o = jt - 4 * t
                    # causal: columns < jt*128 of this query block see none of
                    # this j-tile's keys -> shrink the moving dim
                    lo = max(o, 0) * 128
                    sc = psc_pool.tile([128, SBLK], F32, tag="sc", name="sc")
                    nc.tensor.matmul(
                        sc[:, lo:SBLK],
                        k_sb[h][jt // 4][:, (jt % 4) * 128 : (jt % 4 + 1) * 128],
                        q_sb[h][t][:, lo:SBLK],
                        start=True,
                        stop=True,
                    )
                    ex = ex_pool.tile([128, SBLK], BF16, tag="ex", name="ex")
                    nc.scalar.activation(
                        ex[:, lo:SBLK],
                        sc[:, lo:SBLK],
                        mybir.ActivationFunctionType.Exp,
                        scale=SM_SCALE,
                    )
                    if o >= 0:
                        # partial triangle block: keys > query within it.
                        # DVE (not gpsimd): any gpsimd op triggers library
                        # load/unload drain barriers that stall every engine
                        nc.vector.tensor_mul(
                            ex[:, lo : lo + 128], ex[:, lo : lo + 128], tri_sb[:]
                        )
                    # denominator accumulation on the DVE (off the PE)
                    if jt == 0:
                        ex_first = ex
                    elif jt == 1:
                        nc.vector.tensor_add(
                            acc[:, lo:SBLK], ex_first[:, lo:SBLK], ex[:, lo:SBLK]
                        )
                        if lo > 0:  # only t==0: jt=1 starts at col 128
                            nc.vector.tensor_copy(acc[:, 0:lo], ex_first[:, 0:lo])
                    else:
                        nc.vector.tensor_add(
                            acc[:, lo:SBLK], acc[:, lo:SBLK], ex[:, lo:SBLK]
                        )
                    inflight.append((jt, lo, ex))
                    if len(inflight) > win:
                        emit_ctx(inflight.pop(0))
                    yield
                # yield before the flush: a woven chain covers the last
                # exp->mask latency so the remaining ctx matmuls never stall
                yield
                for work in inflight:
                    emit_ctx(work)
                yield
                # denominator: all-ones matmul = partition-sum broadcast to all
                # 128 partitions in one 213ns PE op (no gpsimd round trip)
                den_b = psc_pool.tile([128, SBLK], F32, tag="sc", name="den")
                nc.tensor.matmul(
                    den_b[:], ones_sb[:], acc[:], start=True, stop=True
                )
                rc = rc_pool.tile([128, SBLK], F32, tag="rc", name="rc")
                nc.vector.reciprocal_approx_fast(out=rc[:], in_=den_b[:])
                nc.vector.tensor_mul(ctx16[h][t][:], ctx_ps[:], rc[:])
                yield

            def emit_out_mchain(pout_pool, sb, m):
                ssl = slice(sb * SBLK, (sb + 1) * SBLK)
                po = pout_pool.tile([128, SBLK], F32, tag="pj", name="po")
                for h in range(HPC):
                    nc.tensor.matmul(
                        po[:],
                        wo_t[h][:, m * 128 : (m + 1) * 128],
                        ctx16[h][sb][:],
                        start=(h == 0),
                        stop=(h == HPC - 1),
                    )
                ot = osb_pool.tile([128, SBLK], F32, tag="ot", name="ot")
                # alternate copy engine so neither queue gates the PE
                if m % 2 == 0:
                    nc.scalar.copy(ot[:], po[:])
                else:
                    nc.vector.tensor_copy(ot[:], po[:])
                nc.sync.dma_start(outT[m * 128 : (m + 1) * 128, ssl], ot[:])

            def advance(gen_list, n):
                # round-robin the first two live rows: a sibling row's j-tiles
                # hide the finalize latency (den->recip->fmul) of the other
                rr = 0
                while n > 0 and gen_list:
                    idx = rr % min(2, len(gen_list))
                    rr += 1
                    try:
                        next(gen_list[idx])
                        n -= 1
                    except StopIteration:
                        gen_list.pop(idx)

            # ------------- phase A: QKV streams with woven attention -------------
            with (
                tc.tile_pool(name="ps_sc", bufs=3, space="PSUM") as psc_pool,
                tc.tile_pool(name="ps_ctx", bufs=2, space="PSUM") as pctx_pool,
                tc.tile_pool(name="ps_j", bufs=3, space="PSUM") as pj_pool,
            ):
                # stream 0 ramp: interleave q0+q1 e-wise so the PE issues two
                # matmuls per arriving (wq[e], x[e]) pair while DMA-fed
                ps0 = pj_pool.tile([128, SBLK], F32, tag="pj", name="pj")
                ps1 = pj_pool.tile([128, SBLK], F32, tag="pj", name="pj")
                for e in range(NECH):
                    for m, ps in ((0, ps0), (1, ps1)):
                        nc.tensor.matmul(
                            ps[:],
                            wq_t[e][:, m * 128 : (m + 1) * 128],
                            xts[0][e][:],
                            start=(e == 0),
                            stop=(e == NECH - 1),
                            skip_group_check=True,
                        )
                rope_to(q_sb[0][0], ps0, slice(0, SBLK))
                rope_to(q_sb[1][0], ps1, slice(0, SBLK))

                for sb in range(NSBLK):
                    rows = (
                        [att_row(h, sb - 1) for h in range(HPC)] if sb >= 1 else []
                    )
                    n_yields = 4 * (4 * sb + 3)
                    per = max(1, -(-n_yields // 12))  # ceil over 12 chains
                    chains = [("q", m) for m in range(HPC)]
                    chains += [("k", m) for m in range(HPC)]
                    chains += [("v", m) for m in range(SBLK // 128)]
                    if sb == 0:
                        chains = chains[2:]  # q0, q1 already emitted above
                    for ci, (kind, m) in enumerate(chains):
                        emit_qkv_chain(pj_pool, sb, kind, m)
                        if ci == 1 and sb < NSBLK - 1:
                            prefetch_x(sb + 1)
                        if ci == 3 and sb == 0:
                            load_wo()
                        advance(rows, per)
                    advance(rows, 10**9)

                # -------- tail: att(3) woven with the output projection --------
                # out-proj chains share the (now idle) projection PSUM pool,
                # avoiding a pool-transition barrier on the tensor queue
                rows = [att_row(h, NSBLK - 1) for h in range(HPC)]
                for sb in range(NSBLK - 1):
                    for m in range(E // 128):
                        emit_out_mchain(pj_pool, sb, m)
                        advance(rows, 2)
                advance(rows, 10**9)
                for m in range(E // 128):
                    emit_out_mchain(pj_pool, NSBLK - 1, m)

    nc.compile()
    return nc


def _rope_tables():
    inv_freq = 1.0 / (10000.0 ** (np.arange(0, D, 2, dtype=np.float64) / D))
    t = np.arange(S, dtype=np.float64)
    freqs = np.outer(t, inv_freq)  # (S, D/2)
    emb = np.concatenate([freqs, freqs], axis=-1)  # (S, D)
    cosT = np.cos(emb).T.astype(np.float32).copy()  # (D, S)
    sinT = np.sin(emb).T.astype(np.float32)
    sinTs = sinT.copy()
    sinTs[: D // 2] = -sinT[: D // 2]
    return cosT, np.ascontiguousarray(sinTs)


def _numpy_fallback(x, mask, wqkv, bqkv, wout, bout):
    qkv = x @ wqkv.T + bqkv
    q, k, v = np.split(qkv, 3, axis=-1)
    q = q.reshape(B, S, H, D).transpose(0, 2, 1, 3)
    k = k.reshape(B, S, H, D).transpose(0, 2, 1, 3)
    v = v.reshape(B, S, H, D).transpose(0, 2, 1, 3)
    inv_freq = 1.0 / (10000.0 ** (np.arange(0, D, 2, dtype=np.float32) / D))
    t = np.arange(S, dtype=np.float32)
    freqs = np.outer(t, inv_freq)
    emb = np.concatenate([freqs, freqs], axis=-1)
    cos, sin = np.cos(emb), np.sin(emb)

    def rot(a):
        a1, a2 = np.split(a, 2, axis=-1)
        return np.concatenate([-a2, a1], axis=-1)

    q = q * cos + rot(q) * sin
    k = k * cos + rot(k) * sin
    scores = np.einsum("bhqd,bhkd->bhqk", q, k) * SM_SCALE
    scores = np.where(mask, -np.inf, scores)
    scores = scores - scores.max(axis=-1, keepdims=True)
    w = np.exp(scores)
    w = w / w.sum(axis=-1, keepdims=True)
    ctx = np.einsum("bhqk,bhkd->bhqd", w, v)
    ctx = ctx.transpose(0, 2, 1, 3).reshape(B, S, E)
    return (ctx @ wout.T + bout).astype(np.float32)


def kernel(x, mask, wqkv, bqkv, wout, bout, **_):
    x = np.ascontiguousarray(np.asarray(x), dtype=np.float32)
    wqkv = np.ascontiguousarray(np.asarray(wqkv), dtype=np.float32)
    bqkv = np.asarray(bqkv, dtype=np.float32)
    wout = np.ascontiguousarray(np.asarray(wout), dtype=np.float32)
    bout = np.asarray(bout, dtype=np.float32)
    mask = np.asarray(mask)

    causal = np.array_equal(mask, np.triu(np.ones((S, S), dtype=bool), k=1))
    if not causal or np.any(bqkv) or ml_dtypes is None:
        return _numpy_fallback(x, mask, wqkv, bqkv, wout, bout)

    if "nc" not in _CACHE:
        _CACHE["nc"] = _build_nc()
    nc = _CACHE["nc"]

    bf16 = ml_dtypes.bfloat16
    cosT, sinTs = _rope_tables()
    cosT = cosT.astype(bf16)
    sinTs = sinTs.astype(bf16)
    r = np.arange(128)
    tri = (r[:, None] <= r[None, :]).astype(bf16)
    ones = np.ones((128, 128), dtype=np.float32)

    in_maps = []
    for c in range(N_CORES):
        b, hg = divmod(c, GPB)
        cols = slice(hg * DPC, (hg + 1) * DPC)
        wq = wqkv[0 * E : 1 * E, :][cols, :]  # (512, E)
        wk = wqkv[1 * E : 2 * E, :][cols, :]
        wv = wqkv[2 * E : 3 * E, :][cols, :]
        in_maps.append(
            {
                "xT": np.ascontiguousarray(x[b].T).astype(bf16),
                "wqkT": np.ascontiguousarray(
                    np.concatenate([wq, wk], axis=0).T
                ).astype(bf16),
                "wvT": np.ascontiguousarray(wv.T).astype(bf16),
                "woutT": np.ascontiguousarray(wout[:, cols].T).astype(bf16),
                "cosT": cosT,
                "sinTs": sinTs,
                "tri": tri,
                "ones": ones,
            }
        )

    res = bass_utils.run_bass_kernel_spmd(
        nc, in_maps, core_ids=list(range(N_CORES)), **_RUN_KWARGS
    )
    _CACHE["last_results"] = res

    out = np.empty((B, S, E), dtype=np.float32)
    for b in range(B):
        acc = res.results[b * GPB]["outT"].copy()
        for g in range(1, GPB):
            acc += res.results[b * GPB + g]["outT"]
        out[b] = acc.T
    out += bout
    return out
